# revision 1
# baseline (speedup 1.0000x reference)
"""Trainium2 Bass kernel for nn_BalancedTreeCell (binary-tree GNN message passing).

Math per batch row (independent per row -> pure data parallel over N=16 rows,
2 rows per NeuronCore on 8 cores):

  state = LN(input @ w_word + b_word)                       [S, D]
  repeat log2(S) times:
     l, r    = state[0::2], state[1::2]
     h       = gelu([l r] @ w1 + b1)                        [S/2, H]
     c       = h @ w2 + b2   -> f1,f2,i = sigmoid(c[:3D]), parent = c[3D:]
     state   = LN(f1*l + f2*r + i*parent)                   [S/2, D]
  out = state[0]                                            [D]

On-chip layout: state kept FEATURE-major ([D partitions, tokens]) in SBUF for
the whole tree (all matmuls contract over features = partitions; even/odd
token selection is a stride-2 free-dim view).  LayerNorm over features
(= partitions) uses PE ones-matmuls for sum/sum-of-squares and K=1 PE
broadcast matmuls to apply per-token scale/shift (with the LN gamma/beta
folded into the broadcast weights).  All matmul operands are float32r
(TRN2 fast fp32 mode: 1 cycle/row at moving-dim >= 256).
input_mask is all-ones per the problem spec, so the mask recursion is the
identity and is skipped.
"""

import numpy as np

import concourse.bass as bass
import concourse.bacc as bacc
import concourse.tile as tile
import concourse.mybir as mybir
from concourse.bass_utils import run_bass_kernel_spmd

F32 = mybir.dt.float32
F32R = mybir.dt.float32r
AF = mybir.ActivationFunctionType
ALU = mybir.AluOpType

P = 128
N_TOT = 16
S_FULL = 4096
D = 256
H = 1024
N_CORES = 8
R = N_TOT // N_CORES          # rows per core
DPT = D // P                  # 2 feature partition-tiles
EPS = 1e-5
G = 512                       # token group (PSUM bank = 512 fp32)

# DVE writing float32r outputs directly (plan A). If the BIR verifier or
# lower_dve rejects it, set False: DVE then writes f32 and ACT re-rounds the
# few tensors that feed f32r matmuls.
DVE_W_F32R = True
PIPE_DEPTH = 1


def _build(S=S_FULL, iters=1, max_levels=None, skip_stage0=False):
    nc = bacc.Bacc("TRN2", target_bir_lowering=False, debug=False)
    T0 = R * S                       # flat tokens entering the tree
    n_levels = int(np.log2(S))

    # ---- DRAM I/O (per core) ----
    x_d = nc.dram_tensor("x", [T0, D], F32R, kind="ExternalInput").ap()
    w1_d = nc.dram_tensor("w1t", [P, 4, H], F32R, kind="ExternalInput").ap()
    w2_d = nc.dram_tensor("w2t", [P, 8, H], F32R, kind="ExternalInput").ap()
    ww_d = nc.dram_tensor("wwt", [P, 2, D], F32R, kind="ExternalInput").ap()
    b1_d = nc.dram_tensor("b1c", [P, 8], F32, kind="ExternalInput").ap()
    b2_d = nc.dram_tensor("b2c", [P, 8], F32, kind="ExternalInput").ap()
    bw_d = nc.dram_tensor("bwc", [P, 2], F32, kind="ExternalInput").ap()
    lnr_d = nc.dram_tensor("lnrow", [4, D], F32R, kind="ExternalInput").ap()
    lnb_d = nc.dram_tensor("lnbcol", [P, 2, 2], F32, kind="ExternalInput").ap()
    ones_d = nc.dram_tensor("onescol", [P, 1], F32R, kind="ExternalInput").ap()
    onesr_d = nc.dram_tensor("onesrow", [1, P], F32R, kind="ExternalInput").ap()
    b1r_d = nc.dram_tensor("b1row", [1, H], F32R, kind="ExternalInput").ap()
    b2r_d = nc.dram_tensor("b2row", [1, H], F32R, kind="ExternalInput").ap()
    gcr_d = nc.dram_tensor("gcrep", [P, D], F32, kind="ExternalInput").ap()
    bcr_d = nc.dram_tensor("bcrep", [P, D], F32, kind="ExternalInput").ap()
    eye_d = nc.dram_tensor("eye", [P, P], F32R, kind="ExternalInput").ap()
    out_d = nc.dram_tensor("out", [R, D], F32, kind="ExternalOutput").ap()

    zdt = F32R if DVE_W_F32R else F32

    with tile.TileContext(nc) as tc:
        cst = tc.alloc_tile_pool(name="cst", bufs=1)
        stp = tc.alloc_tile_pool(name="stp", bufs=1)
        ring_p = tc.alloc_tile_pool(name="ring", bufs=2)
        sb = tc.alloc_tile_pool(name="sb", bufs=2)
        sb3 = tc.alloc_tile_pool(name="sb3", bufs=3)
        hsb = tc.alloc_tile_pool(name="hsb", bufs=1)
        rows = tc.alloc_tile_pool(name="rows", bufs=1)
        ps = tc.alloc_tile_pool(name="ps", bufs=2, space="PSUM")
        ps1 = tc.alloc_tile_pool(name="ps1", bufs=1, space="PSUM")

        # ---- constants ----
        w1s = cst.tile([P, 4, H], F32R)
        nc.sync.dma_start(out=w1s, in_=w1_d)
        w2s = cst.tile([P, 8, H], F32R)
        nc.sync.dma_start(out=w2s, in_=w2_d)
        wws = cst.tile([P, 2, D], F32R)
        nc.sync.dma_start(out=wws, in_=ww_d)
        b1s = cst.tile([P, 8], F32)
        nc.sync.dma_start(out=b1s, in_=b1_d)
        b2s = cst.tile([P, 8], F32)
        nc.sync.dma_start(out=b2s, in_=b2_d)
        bws = cst.tile([P, 2], F32)
        nc.sync.dma_start(out=bws, in_=bw_d)
        lnrs = []
        for i in range(4):
            lt = cst.tile([1, D], F32R, name=f"lnr{i}")
            nc.sync.dma_start(out=lt, in_=lnr_d[i:i + 1, :])
            lnrs.append(lt)
        lnbs = cst.tile([P, 2, 2], F32)
        nc.sync.dma_start(out=lnbs, in_=lnb_d)
        oness = cst.tile([P, 1], F32R)
        nc.sync.dma_start(out=oness, in_=ones_d)
        eyes = cst.tile([P, P], F32R)
        nc.sync.dma_start(out=eyes, in_=eye_d)
        epst = cst.tile([1, 1], F32)
        nc.vector.memset(epst, EPS)
        onesr = cst.tile([1, P], F32R)
        nc.sync.dma_start(out=onesr, in_=onesr_d)
        b1row = cst.tile([1, H], F32R)
        nc.sync.dma_start(out=b1row, in_=b1r_d)
        b2row = cst.tile([1, H], F32R)
        nc.sync.dma_start(out=b2row, in_=b2r_d)
        gcrep = cst.tile([P, D], F32)
        nc.sync.dma_start(out=gcrep, in_=gcr_d)
        bcrep = cst.tile([P, D], F32)
        nc.sync.dma_start(out=bcrep, in_=bcr_d)
        epscol = cst.tile([P, 1], F32)
        nc.vector.memset(epscol, EPS)

        # ---- persistent state buffers (feature-major [P, DPT, tokens]) ----
        TA = T0 // 2
        sA = stp.tile([P, DPT, TA], F32R, tag="sA", name="sA")
        sB = stp.tile([P, DPT, max(TA // 2, 1)], F32R, tag="sB", name="sB")

        def layer_norm(z, zr, Tg, ln, out_aps):
            """z: [P, DPT, Tg] tile (dtype zdt); zr: f32r view of z for PE.
            out_aps[pt]: destination state slices [P, Tg] (f32r)."""
            # z^2 (f32r, for the sum-of-squares matmul)
            if DVE_W_F32R:
                zsq = sb.tile([P, DPT, G], F32R, tag="zsq", name="zsq", bufs=1)[:, :, :Tg]
                nc.scalar.activation(out=zsq, in_=z, func=AF.Square)
            else:
                zsq = sb.tile([P, DPT, G], F32R, tag="zsq", name="zsq")[:, :, :Tg]
                nc.scalar.activation(out=zsq, in_=z, func=AF.Square)
            # stats: row0 = sum(z), row1 = sum(z^2)  (PE ones-matmul)
            st = ps1.tile([1, G], F32, tag="st", name="st")[:, :Tg]
            sq = ps1.tile([1, G], F32, tag="sq", name="sq")[:, :Tg]
            for pt in range(DPT):
                nc.tensor.matmul(st, lhsT=oness, rhs=zr[:, pt, :],
                                 start=(pt == 0), stop=(pt == DPT - 1))
            for pt in range(DPT):
                nc.tensor.matmul(sq, lhsT=oness, rhs=zsq[:, pt, :],
                                 start=(pt == 0), stop=(pt == DPT - 1))
            # per-token chain on [1, Tg] rows (ex2 reused as veps, mq as scratch)
            mu = rows.tile([1, G], F32, tag="mu", name="mu")[:, :Tg]
            nc.vector.tensor_scalar_mul(mu, st, 1.0 / D)
            ex2 = rows.tile([1, G], F32, tag="ex2", name="ex2")[:, :Tg]
            nc.vector.tensor_scalar_mul(ex2, sq, 1.0 / D)
            mq = rows.tile([1, G], F32, tag="mq", name="mq")[:, :Tg]
            nc.vector.tensor_mul(mq, mu, mu)
            nc.vector.scalar_tensor_tensor(
                out=ex2, in0=ex2, scalar=EPS, in1=mq,
                op0=ALU.add, op1=ALU.subtract)
            rvar = rows.tile([1, G], F32, tag="rvar", name="rvar")[:, :Tg]
            nc.vector.reciprocal_approx_accurate(rvar, ex2, mq)
            rsig = rows.tile([1, G], F32R, tag="rsig", name="rsig")[:, :Tg]
            nc.scalar.activation(out=rsig, in_=rvar, func=AF.Sqrt)
            ms = rows.tile([1, G], F32R if DVE_W_F32R else F32, tag="ms", name="ms")[:, :Tg]
            nc.vector.tensor_mul(ms, mu, rsig)
            if not DVE_W_F32R:
                msr = rows.tile([1, G], F32R, tag="msr", name="msr")[:, :Tg]
                nc.scalar.copy(out=msr, in_=ms)
                ms = msr
            # broadcast + apply:  out = (z * (g x rsig) + b) + (-g x mu*rsig)
            for pt in range(DPT):
                b1p = ps.tile([P, G], F32, tag="bc", name="bc")[:, :Tg]
                nc.tensor.matmul(b1p, lhsT=lnrs[2 * ln][:, pt * P:(pt + 1) * P],
                                 rhs=rsig, start=True, stop=True)
                b2p = ps.tile([P, G], F32, tag="bc", name="bc")[:, :Tg]
                nc.tensor.matmul(b2p, lhsT=lnrs[2 * ln + 1][:, pt * P:(pt + 1) * P],
                                 rhs=ms, start=True, stop=True)
                t = sb.tile([P, G], F32, tag="tap", name="tap")[:, :Tg]
                nc.vector.tensor_mul(t, z[:, pt, :], b1p)
                if DVE_W_F32R:
                    nc.vector.scalar_tensor_tensor(
                        out=out_aps[pt], in0=t, scalar=lnbs[:, ln, pt:pt + 1],
                        in1=b2p, op0=ALU.add, op1=ALU.add)
                else:
                    t2 = sb.tile([P, G], F32, tag="tap2", name="tap2")[:, :Tg]
                    nc.vector.scalar_tensor_tensor(
                        out=t2, in0=t, scalar=lnbs[:, ln, pt:pt + 1],
                        in1=b2p, op0=ALU.add, op1=ALU.add)
                    nc.scalar.copy(out=out_aps[pt], in_=t2)

        def cell_group(xk, lr, Tg, out_aps):
            """One 512-token group of the tree cell (emits M phase, returns
            the LN phase as a closure).
            xk: 4 rhs k-tile APs [P, Tg] (f32r) = [l0, l1, r0, r1]
            lr: (l0, l1, r0, r1) same APs for the elementwise combine
            out_aps[pt]: state output slices."""
            # mm1 + gelu -> h
            h = hsb.tile([P, 8, G], F32R, tag="h", name="h")[:, :, :Tg]
            for m in range(8):
                ph = ps.tile([P, G], F32, tag="ph", name="ph")[:, :Tg]
                for k in range(4):
                    nc.tensor.matmul(ph, lhsT=w1s[:, k, m * P:(m + 1) * P],
                                     rhs=xk[k], start=(k == 0), stop=(k == 3))
                nc.scalar.activation(out=h[:, m, :], in_=ph, func=AF.Gelu,
                                     bias=b1s[:, m:m + 1])
            # mm2 -> c tiles; combine z = f1*l + f2*r + i*parent
            z = sb.tile([P, DPT, G], zdt, tag="z", name="z")[:, :, :Tg]
            par = [None, None]
            gp_add = [None, None]
            gp_add2 = [None, None]
            for m2 in (6, 7, 0, 1, 2, 3, 4, 5):
                pc = ps.tile([P, G], F32, tag="pc", name="pc")[:, :Tg]
                for k in range(8):
                    nc.tensor.matmul(pc, lhsT=w2s[:, k, m2 * P:(m2 + 1) * P],
                                     rhs=h[:, k, :], start=(k == 0), stop=(k == 7))
                part, pt = divmod(m2, 2)
                if part == 3:
                    pr = sb.tile([P, G], F32R, tag=f"par{pt}", name=f"par{pt}", bufs=1)[:, :Tg]
                    nc.scalar.activation(out=pr, in_=pc, func=AF.Identity,
                                         bias=b2s[:, m2:m2 + 1])
                    par[pt] = pr
                else:
                    gt = sb3.tile([P, G], F32R, tag="gate", name="gate", bufs=2)[:, :Tg]
                    nc.scalar.activation(out=gt, in_=pc, func=AF.Sigmoid,
                                         bias=b2s[:, m2:m2 + 1])
                    if part == 0:
                        nc.vector.tensor_mul(z[:, pt, :], gt, lr[pt])
                    elif part == 1:
                        # f2 term on the otherwise-idle GPSIMD engine
                        tg_ = sb.tile([P, G], zdt, tag="tg", name="tg", bufs=1)[:, :Tg]
                        nc.gpsimd.tensor_mul(tg_, gt, lr[2 + pt])
                        gp_add[pt] = tg_
                    else:
                        # i term product also on GPSIMD; adds stay on DVE
                        tb = sb.tile([P, G], zdt, tag="tb", name="tb", bufs=1)[:, :Tg]
                        nc.gpsimd.tensor_mul(tb, gt, par[pt])
                        gp_add2[pt] = tb
            for pt in range(DPT):
                nc.vector.tensor_add(z[:, pt, :], z[:, pt, :], gp_add[pt])
                nc.vector.tensor_add(z[:, pt, :], z[:, pt, :], gp_add2[pt])
            zr = z if DVE_W_F32R else None
            if not DVE_W_F32R:
                zr = sb.tile([P, DPT, G], F32R, tag="zrc", name="zrc")[:, :, :Tg]
                nc.scalar.copy(out=zr, in_=z)
            return lambda: layer_norm(z, zr, Tg, 1, out_aps)

        def cell_group_tm(xk, Tg, out_aps):
            """Token-major cell for tiny levels (Tg <= 128 output tokens):
            weights are the moving operand (19 matmuls instead of 96) and
            LayerNorm uses bn_stats along the free dim."""
            # mm1: h[t, :] = gelu(cat @ w1 + b1), two 512-col halves
            h_tm = sb.tile([P, 2, 512], F32R, tag="h", name="htm")[:Tg]
            for nh in range(2):
                hp = ps.tile([P, 512], F32, tag="ph", name="php")[:Tg]
                for k in range(4):
                    nc.tensor.matmul(hp, lhsT=xk[k],
                                     rhs=w1s[:, k, nh * 512:(nh + 1) * 512],
                                     start=(k == 0), stop=False)
                nc.tensor.matmul(hp, lhsT=onesr[:, :Tg],
                                 rhs=b1row[:, nh * 512:(nh + 1) * 512],
                                 start=False, stop=True)
                nc.scalar.activation(out=h_tm[:, nh, :], in_=hp, func=AF.Gelu)
            # hT [128, 8, Tg] for mm2 lhsT
            hT = sb.tile([P, 8, P], F32R, tag="x0", name="hT", bufs=1)[:, :, :Tg]
            for half in range(2):
                tp = ps.tile([P, 4, P], F32R, tag="bc", name="tpbc")[:, :, :Tg]
                for j in range(4):
                    nc.tensor.transpose(tp[:, j, :],
                                        h_tm[:, half, j * P:(j + 1) * P],
                                        eyes[:Tg, :Tg])
                nc.scalar.copy(out=hT[:, half * 4:(half + 1) * 4, :], in_=tp)
            # l, r token-major [Tg, 512] = [l | r]
            lrp = ps.tile([P, 512], F32R, tag="bc", name="lrp")[:Tg]
            for i4 in range(4):
                nc.tensor.transpose(lrp[:, i4 * P:(i4 + 1) * P], xk[i4], eyes)
            lr_tm = sb.tile([P, 512], F32R, tag="tb", name="lrtm", bufs=1)[:Tg]
            nc.scalar.copy(out=lr_tm, in_=lrp)
            # mm2: c[t, 1024] = h @ w2 + b2, half1 (i|parent) first
            csb = sb.tile([P, 1024], F32R, tag="zsq", name="csb", bufs=1)[:Tg]
            for nh in (1, 0):
                cp = ps.tile([P, 512], F32, tag="pc", name="cp2")[:Tg]
                for k in range(8):
                    nc.tensor.matmul(cp, lhsT=hT[:, k, :],
                                     rhs=w2s[:, k, nh * 512:(nh + 1) * 512],
                                     start=(k == 0), stop=False)
                nc.tensor.matmul(cp, lhsT=onesr[:, :Tg],
                                 rhs=b2row[:, nh * 512:(nh + 1) * 512],
                                 start=False, stop=True)
                if nh == 1:
                    nc.scalar.activation(out=csb[:, 512:768], in_=cp[:, 0:256],
                                         func=AF.Sigmoid)
                    nc.scalar.copy(out=csb[:, 768:1024], in_=cp[:, 256:512])
                else:
                    nc.scalar.activation(out=csb[:, 0:512], in_=cp,
                                         func=AF.Sigmoid)
            # combine + LN (token-major)
            zt = sb.tile([P, 2, D], F32R, tag="tap", name="ztm")[:Tg]
            z_ = zt[:, 0, :]
            tmp_ = zt[:, 1, :]
            nc.vector.tensor_mul(z_, csb[:, 512:768], csb[:, 768:1024])
            nc.vector.tensor_mul(tmp_, csb[:, 0:256], lr_tm[:, 0:256])
            nc.vector.tensor_add(z_, z_, tmp_)
            nc.vector.tensor_mul(tmp_, csb[:, 256:512], lr_tm[:, 256:512])
            nc.vector.tensor_add(z_, z_, tmp_)
            bst = rows.tile([P, 6], F32, tag="mu", name="bst")[:Tg]
            nc.vector.bn_stats(out=bst, in_=z_)
            mv = rows.tile([P, 2], F32, tag="ex2", name="mv")[:Tg]
            nc.vector.bn_aggr(out=mv, in_=bst)
            sg = rows.tile([P, 1], F32, tag="mq", name="sg")[:Tg]
            nc.scalar.activation(out=sg, in_=mv[:, 1:2], func=AF.Sqrt,
                                 bias=epscol[:Tg])
            rs = rows.tile([P, 1], F32, tag="rvar", name="rs")[:Tg]
            nc.vector.reciprocal(out=rs, in_=sg)
            zl = sb3.tile([P, D], F32R, tag="gate", name="zl", bufs=2)[:Tg]
            nc.vector.tensor_scalar(out=zl, in0=z_, scalar1=mv[:, 0:1],
                                    scalar2=rs, op0=ALU.subtract, op1=ALU.mult)
            nc.vector.tensor_mul(zl, zl, gcrep[:Tg])
            nc.vector.tensor_add(zl, zl, bcrep[:Tg])
            for pt in range(DPT):
                zp = ps.tile([P, P], F32R, tag="bc", name="zp")[:, :Tg]
                nc.tensor.transpose(zp, zl[:, pt * P:(pt + 1) * P],
                                    eyes[:Tg, :Tg])
                nc.scalar.copy(out=out_aps[pt], in_=zp)

        def body():
            # ---- stage 0: load + transpose + word-linear + LN0 ----
            TR = min(1024, T0)               # ring tile tokens
            n_ring = T0 // TR
            nsub = TR // 512 if TR >= 512 else 1
            sub_t = min(512, TR)
            xr = x_d.rearrange("(a s p) d -> a p s d", p=P, s=max(sub_t // P, 1))

            def stage0_sub(rt, g, sub):
                if True:
                    gi = g * nsub + sub
                    itm = sb3.tile([P, max(sub_t // P, 1), D], F32R, tag="itm", name="itm", bufs=2)
                    nc.sync.dma_start(out=itm, in_=xr[gi])
                    # transpose to feature-major x0 [P, DPT, sub_t]
                    x0 = sb.tile([P, DPT, 512], F32R, tag="x0", name="x0", bufs=1)[:, :, :sub_t]
                    for pt in range(DPT):
                        xtp = ps.tile([P, 512], F32R, tag="ph", name="ph")[:, :sub_t]
                        for s in range(max(sub_t // P, 1)):
                            nc.tensor.transpose(
                                xtp[:, s * P:(s + 1) * P],
                                itm[:, s, pt * P:(pt + 1) * P], eyes)
                        nc.scalar.copy(out=x0[:, pt, :], in_=xtp)
                    # word linear
                    z0 = sb.tile([P, DPT, 512], F32R, tag="z", name="z")[:, :, :sub_t]
                    for pt in range(DPT):
                        pw = ps.tile([P, 512], F32, tag="pc", name="pc")[:, :sub_t]
                        for k in range(DPT):
                            nc.tensor.matmul(pw, lhsT=wws[:, k, pt * P:(pt + 1) * P],
                                             rhs=x0[:, k, :],
                                             start=(k == 0), stop=(k == DPT - 1))
                        nc.scalar.activation(out=z0[:, pt, :], in_=pw,
                                             func=AF.Identity, bias=bws[:, pt:pt + 1])
                    return lambda: layer_norm(
                        z0, z0, sub_t, 0,
                        [rt[:, pt, sub * sub_t:(sub + 1) * sub_t]
                         for pt in range(DPT)])

            # ---- software pipeline: LN phase trails one unit behind M.
            # M phases may only be emitted once every LN they read from has
            # been emitted (emission order defines dataflow under Tile), so
            # each M declares the newest unit id it depends on.
            pending = []   # [(uid, ln_fn)]
            uid_ctr = [0]

            def flush_through(uid):
                while pending and pending[0][0] <= uid:
                    pending.pop(0)[1]()

            def flush_all():
                while pending:
                    pending.pop(0)[1]()

            def mstep(m_fn, after=None):
                if after is not None:
                    flush_through(after)
                ln_fn = m_fn()
                flush_all()
                uid_ctr[0] += 1
                pending.append((uid_ctr[0], ln_fn))
                if PIPE_DEPTH == 0:
                    flush_all()
                return uid_ctr[0]

            # stage0 runs one ring-group ahead of level-1
            Tg1 = TR // 2
            ring = [None] * (n_ring + 1)
            s0_uid = [None] * n_ring

            def s0(g):
                if g < n_ring:
                    rt = ring_p.tile([P, DPT, TR], F32R, tag="ring",
                                     name="ring")
                    ring[g] = rt
                    for sub in range(nsub):
                        s0_uid[g] = mstep(
                            lambda rt=rt, g=g, sub=sub: stage0_sub(rt, g, sub))

            def l1(g):
                rt = ring[g]
                xk = [rt[:, 0, 0:2 * Tg1:2], rt[:, 1, 0:2 * Tg1:2],
                      rt[:, 0, 1:2 * Tg1:2], rt[:, 1, 1:2 * Tg1:2]]
                out_aps = [sA[:, pt, g * Tg1:(g + 1) * Tg1]
                           for pt in range(DPT)]
                mstep(lambda: cell_group(xk, xk, Tg1, out_aps),
                      after=s0_uid[g])

            s0(0)
            for g in range(n_ring):
                s0(g + 1)
                l1(g)

            # ---- remaining tree levels ----
            prev, cur = sA, sB
            Tin = T0 // 2
            for lev in range(1, n_levels if max_levels is None
                             else min(n_levels, max_levels)):
                To = Tin // 2
                ngroups = max(To // G, 1)
                Tg = min(G, To)
                flush_all()   # next level reads every group of prev level
                for g in range(ngroups):
                    base = 2 * g * G
                    # k-tile order [l0, l1, r0, r1]
                    xk = [prev[:, 0, base:base + 2 * Tg:2],
                          prev[:, 1, base:base + 2 * Tg:2],
                          prev[:, 0, base + 1:base + 2 * Tg:2],
                          prev[:, 1, base + 1:base + 2 * Tg:2]]
                    out_aps = [cur[:, pt, g * G:g * G + Tg] for pt in range(DPT)]
                    if Tg <= 128:
                        cell_group_tm(xk, Tg, out_aps)
                    else:
                        mstep(lambda xk=xk, Tg=Tg, out_aps=out_aps:
                              cell_group(xk, xk, Tg, out_aps))
                prev, cur = cur, prev
                Tin = To
            flush_all()

            # ---- emit output [R, D] (prev holds the final 2 tokens) ----
            outt = sb.tile([R, D], F32, tag="outt", name="outt")
            for pt in range(DPT):
                otp = ps1.tile([R, P], F32R, tag="st", name="otp")
                nc.tensor.transpose(otp, prev[:, pt, 0:R], eyes)
                nc.vector.tensor_copy(out=outt[:, pt * P:(pt + 1) * P], in_=otp)
            nc.sync.dma_start(out=out_d, in_=outt)

        if iters == 1:
            body()
        else:
            with tc.For_i(0, iters, 1):
                body()

        for p_ in (ps1, ps, rows, hsb, sb3, sb, ring_p, stp, cst):
            p_.release()

    nc.compile()
    return nc


def _prep_weights(w_word, b_word, w1, bias1, w2, bias2,
                  ln0_g, ln0_b, lnc_g, lnc_b):
    f = np.float32
    w1h = np.ascontiguousarray(w1.reshape(4, P, H).transpose(1, 0, 2), dtype=f)
    w2h = np.ascontiguousarray(w2.reshape(8, P, H).transpose(1, 0, 2), dtype=f)
    wwh = np.ascontiguousarray(w_word.reshape(2, P, D).transpose(1, 0, 2), dtype=f)
    b1h = np.ascontiguousarray(bias1.reshape(8, P).T, dtype=f)
    b2h = np.ascontiguousarray(bias2.reshape(8, P).T, dtype=f)
    bwh = np.ascontiguousarray(b_word.reshape(2, P).T, dtype=f)
    lnrow = np.stack([ln0_g, -ln0_g, lnc_g, -lnc_g]).astype(f)    # [4, D]
    lnbcol = np.ascontiguousarray(
        np.stack([ln0_b, lnc_b]).reshape(2, 2, P).transpose(2, 0, 1), dtype=f)
    return dict(w1t=w1h, w2t=w2h, wwt=wwh, b1c=b1h, b2c=b2h, bwc=bwh,
                lnrow=lnrow, lnbcol=lnbcol,
                onescol=np.ones((P, 1), f), eye=np.eye(P, dtype=f),
                onesrow=np.ones((1, P), f),
                b1row=np.ascontiguousarray(bias1.reshape(1, H), dtype=f),
                b2row=np.ascontiguousarray(bias2.reshape(1, H), dtype=f),
                gcrep=np.broadcast_to(lnc_g, (P, D)).astype(f),
                bcrep=np.broadcast_to(lnc_b, (P, D)).astype(f))


_NC_CACHE = {}


def _get_nc(S=S_FULL, iters=1, max_levels=None, skip_stage0=False):
    key = (S, iters, max_levels, skip_stage0)
    if key not in _NC_CACHE:
        _NC_CACHE[key] = _build(S, iters, max_levels, skip_stage0)
    return _NC_CACHE[key]


def kernel(input, input_mask, w_word, b_word, w1, bias1, w2, bias2,
           ln0_g, ln0_b, lnc_g, lnc_b, _iters=1, _max_levels=None,
           _skip_stage0=False):
    inp = np.asarray(input, dtype=np.float32)
    shared = _prep_weights(
        np.asarray(w_word), np.asarray(b_word), np.asarray(w1),
        np.asarray(bias1), np.asarray(w2), np.asarray(bias2),
        np.asarray(ln0_g), np.asarray(ln0_b), np.asarray(lnc_g),
        np.asarray(lnc_b))
    S = inp.shape[1]
    nc = _get_nc(S, _iters, _max_levels, _skip_stage0)
    in_maps = []
    for c in range(N_CORES):
        m = dict(shared)
        m["x"] = np.ascontiguousarray(
            inp[c * R:(c + 1) * R].reshape(R * S, D))
        in_maps.append(m)
    res = run_bass_kernel_spmd(nc, in_maps, core_ids=list(range(N_CORES)))
    return np.concatenate([res.results[c]["out"] for c in range(N_CORES)],
                          axis=0)

